# revision 3
# baseline (speedup 1.0000x reference)
"""AttentionBlock (GroupNorm + single-head NxN attention + residual) on 8 TRN2 cores.

Data-parallel: batch dim (B=8) sharded 1 batch-image per NeuronCore. Each core
runs the full block for its image:

  x (C=256, N=4096) -> GroupNorm(8 groups) -> qkv = W_qkv @ xn (fp32r) ->
  q,k,v cast to fp8e4m3; attention runs entirely in fp8 DoubleRow matmuls
  (contraction 256 = full C or an m-pair of 128x2 keys in ONE matmul at
  2 MACs/cell/cycle):
    sT = k^T q  (one DR matmul per key-tile), e = exp(sT/16 - 4)  -> fp8
    attnout_u = v @ e (DR, m-pairs), den = ones @ e (DR)
    proj_u = W_out @ attnout_u (fp32r); out = proj_u * (1/den) + b_out + x

The softmax is unnormalized with a constant exponent shift of -4 (keeps
exp() under the fp8e4m3 max of 240; the shift and the 1/den normalization
both commute through the output projection and cancel).
"""

import sys

if "/opt/trn_rl_repo" not in sys.path:
    sys.path.insert(0, "/opt/trn_rl_repo")

import numpy as np

import concourse.bass as bass
import concourse.bacc as bacc
import concourse.tile as tile
import concourse.mybir as mybir
from concourse import bass_utils

# Problem dims (hardcoded per spec)
B, C, HH, WW = 8, 256, 64, 64
N = HH * WW            # 4096
G = 8                  # groupnorm groups
GSZ = C // G           # 32 channels/group
EPS = 1e-5
P = 128                # SBUF partitions
CT = C // P            # 2 channel tiles
NCH = 512              # query-chunk width (free dim per matmul)
NNCH = N // NCH        # 8
MT = N // P            # 32 key tiles
MP = MT // 2           # 16 key-tile pairs
SCALE = 1.0 / np.sqrt(C)
ESHIFT = -4.0          # exponent shift; cancels in normalization
INV_CNT = 1.0 / (GSZ * N)

F32 = mybir.dt.float32
F32R = mybir.dt.float32r
FP8 = mybir.dt.float8e4
DR = mybir.MatmulPerfMode.DoubleRow


def _emit(tc, d, out_d):
    from contextlib import ExitStack

    nc = tc.nc
    AF = mybir.ActivationFunctionType
    OP = mybir.AluOpType
    AX = mybir.AxisListType.X
    ts, ds = bass.ts, bass.ds

    with ExitStack() as ctx:
        const = ctx.enter_context(tc.tile_pool(name="const", bufs=1))
        big = ctx.enter_context(tc.tile_pool(name="big", bufs=1))
        work = ctx.enter_context(tc.tile_pool(name="work", bufs=3))
        small = ctx.enter_context(tc.tile_pool(name="small", bufs=4))
        outp = ctx.enter_context(tc.tile_pool(name="outp", bufs=3))
        psS = ctx.enter_context(tc.tile_pool(name="psS", bufs=3, space="PSUM"))
        psP = ctx.enter_context(tc.tile_pool(name="psP", bufs=2, space="PSUM"))
        psA = ctx.enter_context(tc.tile_pool(name="psA", bufs=1, space="PSUM"))
        psD = ctx.enter_context(tc.tile_pool(name="psD", bufs=1, space="PSUM"))

        # ---------------- load x first (chunked; stats pipeline behind DMA) --
        NC4 = 4                      # head chunks per channel-tile
        CW = N // NC4                # 1024 columns per chunk
        x_d = d["x"]
        x_sb = big.tile([P, CT, N], F32, name="x_sb")
        for t in range(CT):
            for c in range(NC4):
                eng = nc.sync if (t * NC4 + c) % 2 == 0 else nc.gpsimd
                eng.dma_start(out=x_sb[:, t, ds(c * CW, CW)],
                              in_=x_d[ts(t, P), ds(c * CW, CW)])

        # ---------------- constants / weights to SBUF ----------------
        wq_sb = const.tile([P, CT, C], F32R, name="wq_sb")
        wk_sb = const.tile([P, CT, C], F32R, name="wk_sb")
        wv_sb = const.tile([P, CT, C], F32R, name="wv_sb")
        wo_sb = const.tile([P, CT, C], F32R, name="wo_sb")
        for sb, dr in ((wq_sb, d["wq_t"]), (wk_sb, d["wk_t"]),
                       (wv_sb, d["wv_t"]), (wo_sb, d["wo_t"])):
            for ch in range(CT):
                nc.sync.dma_start(out=sb[:, ch, :], in_=dr[ts(ch, P), :])

        bq_sb = const.tile([P, CT], F32, name="bq_sb")
        bk_sb = const.tile([P, CT], F32, name="bk_sb")
        bo_sb = const.tile([P, CT], F32, name="bo_sb")
        gw_sb = const.tile([P, CT], F32, name="gw_sb")
        gb_sb = const.tile([P, CT], F32, name="gb_sb")
        for sb, dr in ((bq_sb, d["b_q"]), (bk_sb, d["b_k"]), (bo_sb, d["b_o"]),
                       (gw_sb, d["gn_w"]), (gb_sb, d["gn_b"])):
            for t in range(CT):
                nc.sync.dma_start(out=sb[:, t:t + 1], in_=dr[t])

        fm_sb = const.tile([P, CT, G], F32, name="fm_sb")
        bm_sb = const.tile([G, CT, P], F32, name="bm_sb")
        for t in range(CT):
            nc.sync.dma_start(out=fm_sb[:, t, :], in_=d["fmask"][t])
            nc.sync.dma_start(out=bm_sb[:, t, :], in_=d["bmask"][t])

        # fp8 ones pair for the den matmul (pair-dim stride kept at 16B)
        ones_sb = const.tile([P, CT, 16], FP8, name="ones_sb")
        nc.vector.memset(ones_sb, 1.0)
        zero_sb = const.tile([P, 1], F32, name="zero_sb")
        nc.vector.memset(zero_sb, 0.0)
        esh_sb = const.tile([P, 1], F32, name="esh_sb")
        nc.vector.memset(esh_sb, ESHIFT)
        eps_sb = const.tile([G, 1], F32, name="eps_sb")
        nc.vector.memset(eps_sb, EPS)

        # ---------------- GroupNorm ----------------
        xn_sb = big.tile([P, CT, N], F32R, name="xn_sb")
        stat = small.tile([P, CT, NC4, 2], F32, name="stat")
        for t in range(CT):
            for c in range(NC4):
                csl = ds(c * CW, CW)
                nc.vector.reduce_sum(out=stat[:, t, c, 0:1], in_=x_sb[:, t, csl],
                                     axis=AX)
                # x^2 into xn (scratch; overwritten below), row-sum into stat
                nc.scalar.activation(out=xn_sb[:, t, csl], in_=x_sb[:, t, csl],
                                     func=AF.Square, bias=zero_sb,
                                     accum_out=stat[:, t, c, 1:2])
                # PE warm-up during the head: slow fp32 matmul on the chunk
                warm = psS.tile([1, NCH], F32, tag="s", name="warm")
                nc.tensor.matmul(warm, lhsT=zero_sb,
                                 rhs=x_sb[:, t, ds(c * CW, NCH)],
                                 start=True, stop=True)

        gps = psS.tile([G, 2], F32, tag="s", name="gps")
        first = True
        for t in range(CT):
            for c in range(NC4):
                nc.tensor.matmul(gps, lhsT=fm_sb[:, t, :], rhs=stat[:, t, c, :],
                                 start=first, stop=(t == CT - 1 and c == NC4 - 1))
                first = False
        grp = small.tile([G, 2], F32, name="grp")    # [mean, rstd]
        gtmp = small.tile([G, 3], F32, name="gtmp")
        nc.vector.tensor_scalar_mul(out=grp[:, 0:1], in0=gps[:, 0:1], scalar1=INV_CNT)
        nc.vector.tensor_scalar_mul(out=gtmp[:, 0:1], in0=gps[:, 1:2], scalar1=INV_CNT)
        nc.vector.tensor_mul(out=gtmp[:, 1:2], in0=grp[:, 0:1], in1=grp[:, 0:1])
        nc.vector.tensor_sub(out=gtmp[:, 2:3], in0=gtmp[:, 0:1], in1=gtmp[:, 1:2])
        nc.scalar.activation(out=gtmp[:, 2:3], in_=gtmp[:, 2:3], func=AF.Sqrt,
                             bias=eps_sb)
        nc.vector.reciprocal(out=grp[:, 1:2], in_=gtmp[:, 2:3])

        ab = small.tile([P, CT, 2], F32, name="ab")  # per-channel scale a, bias b
        for t in range(CT):
            cps = psS.tile([P, 2], F32, tag="s", name="cps")
            nc.tensor.matmul(cps, lhsT=bm_sb[:, t, :], rhs=grp, start=True, stop=True)
            nc.vector.tensor_mul(out=ab[:, t, 0:1], in0=cps[:, 1:2], in1=gw_sb[:, t:t + 1])
            nc.vector.tensor_mul(out=ab[:, t, 1:2], in0=cps[:, 0:1], in1=ab[:, t, 0:1])
            nc.vector.tensor_sub(out=ab[:, t, 1:2], in0=gb_sb[:, t:t + 1], in1=ab[:, t, 1:2])
            nc.vector.tensor_scalar(out=xn_sb[:, t, :], in0=x_sb[:, t, :],
                                    scalar1=ab[:, t, 0:1], scalar2=ab[:, t, 1:2],
                                    op0=OP.mult, op1=OP.add)

        # ---------------- QKV projections (fp32r matmul -> fp8 store) -------
        q_sb = big.tile([P, CT, N], FP8, name="q_sb")   # (c_half, n)
        k_sb = big.tile([P, CT, N], FP8, name="k_sb")
        vT_sb = big.tile([P, MT, C], FP8, name="vT_sb")  # (n, c), n on partitions

        qki = 0
        for (w_sb, b_sb, o_sb) in ((wq_sb, bq_sb, q_sb), (wk_sb, bk_sb, k_sb)):
            for tq in range(CT):
                for nch in range(NNCH):
                    ps = psS.tile([P, NCH], F32, tag="s", name="psqk")
                    for ch in range(CT):
                        nc.tensor.matmul(
                            ps,
                            lhsT=w_sb[:, ch, ts(tq, P)],
                            rhs=xn_sb[:, ch, ds(nch * NCH, NCH)],
                            start=(ch == 0), stop=(ch == CT - 1))
                    osl = o_sb[:, tq, ds(nch * NCH, NCH)]
                    if qki % 2 == 0:
                        nc.vector.tensor_scalar_add(out=osl, in0=ps,
                                                    scalar1=b_sb[:, tq:tq + 1])
                    else:
                        nc.scalar.activation(out=osl, in_=ps, func=AF.Identity,
                                             bias=b_sb[:, tq:tq + 1])
                    qki += 1

        for mt in range(MT):
            ps = psP.tile([P, C], F32, tag="proj", name="psv")
            for ch in range(CT):
                nc.tensor.matmul(ps,
                                 lhsT=xn_sb[:, ch, ts(mt, P)],
                                 rhs=wv_sb[:, ch, :],
                                 start=(ch == 0), stop=(ch == CT - 1))
            # b_v is folded into b_out host-side (softmax rows sum to 1)
            if mt % 2 == 0:
                nc.vector.tensor_copy(out=vT_sb[:, mt, :], in_=ps)
            else:
                nc.scalar.copy(out=vT_sb[:, mt, :], in_=ps)

        # ---------------- attention + output projection (fp8 DoubleRow) -----
        for nch in range(NNCH):
            nsl = ds(nch * NCH, NCH)
            attn = psA.tile([P, CT, NCH], F32, tag="attn", name="attn")
            den = psD.tile([1, NCH], F32, tag="den", name="den")
            for mp in range(MP):
                e = work.tile([P, 2, NCH], FP8, tag="e", name="e", bufs=4)
                for half in range(2):
                    mt = 2 * mp + half
                    s = psS.tile([P, NCH], F32, tag="s", name="s")
                    nc.tensor.matmul(s,
                                     lhsT=k_sb[:, :, ts(mt, P)],
                                     rhs=q_sb[:, :, nsl],
                                     start=True, stop=True, perf_mode=DR)
                    nc.scalar.activation(out=e[:, half, :], in_=s, func=AF.Exp,
                                         bias=esh_sb, scale=SCALE)
                for ch in range(CT):
                    nc.tensor.matmul(attn[:, ch, :],
                                     lhsT=vT_sb[:, ds(2 * mp, 2), ts(ch, P)],
                                     rhs=e,
                                     start=(mp == 0), stop=(mp == MP - 1),
                                     perf_mode=DR)
                nc.tensor.matmul(den, lhsT=ones_sb[:, :, 0:1], rhs=e,
                                 start=(mp == 0), stop=(mp == MP - 1),
                                 perf_mode=DR)

            den_sb = small.tile([1, NCH], F32, tag="den_sb", name="den_sb", bufs=2)
            nc.vector.tensor_copy(out=den_sb, in_=den)
            rden = small.tile([1, NCH], F32, tag="rden", name="rden", bufs=2)
            rscr = small.tile([1, NCH], F32, tag="rscr", name="rscr", bufs=2)
            nc.vector.reciprocal_approx_accurate(out=rden, in_=den_sb, scratch=rscr)
            rdenb = outp.tile([P, NCH], F32, tag="rdenb", name="rdenb", bufs=2)
            nc.gpsimd.partition_broadcast(rdenb, rden)

            atts = outp.tile([P, CT, NCH], F32R, tag="att", name="atts", bufs=2)
            nc.vector.tensor_copy(out=atts, in_=attn)

            for co in range(CT):
                pj = psP.tile([P, NCH], F32, tag="proj", name="pj")
                for ch in range(CT):
                    nc.tensor.matmul(pj,
                                     lhsT=wo_sb[:, ch, ts(co, P)],
                                     rhs=atts[:, ch, :],
                                     start=(ch == 0), stop=(ch == CT - 1))
                f = outp.tile([P, NCH], F32, tag="fout", name="f", bufs=2)
                nc.vector.tensor_tensor(out=f, in0=pj, in1=rdenb, op=OP.mult)
                nc.vector.scalar_tensor_tensor(out=f, in0=f, scalar=bo_sb[:, co:co + 1],
                                               in1=x_sb[:, co, nsl],
                                               op0=OP.add, op1=OP.add)
                nc.sync.dma_start(out=out_d[ts(co, P), nsl], in_=f)


def build_program():
    nc = bacc.Bacc("TRN2", target_bir_lowering=False, debug=False, num_devices=B)
    d = {}

    def din(name, shape, dt_=F32):
        d[name] = nc.dram_tensor(name, list(shape), dt_, kind="ExternalInput").ap()

    din("x", (C, N))
    din("wq_t", (C, C), F32R)
    din("wk_t", (C, C), F32R)
    din("wv_t", (C, C), F32R)
    din("wo_t", (C, C), F32R)
    din("b_q", (CT, P, 1))
    din("b_k", (CT, P, 1))
    din("b_o", (CT, P, 1))
    din("gn_w", (CT, P, 1))
    din("gn_b", (CT, P, 1))
    din("fmask", (CT, P, G))
    din("bmask", (CT, G, P))
    out_d = nc.dram_tensor("out", [C, N], F32, kind="ExternalOutput").ap()

    with tile.TileContext(nc) as tc:
        _emit(tc, d, out_d)
    nc.compile()
    return nc


_PROG = None


def _get_program():
    global _PROG
    if _PROG is None:
        _PROG = build_program()
    return _PROG


def make_in_maps(inputs):
    x = np.ascontiguousarray(np.asarray(inputs["x"], dtype=np.float32))
    w_qkv = np.asarray(inputs["w_qkv"], dtype=np.float32)
    b_qkv = np.asarray(inputs["b_qkv"], dtype=np.float32)
    w_out = np.asarray(inputs["w_out"], dtype=np.float32)
    b_out = np.asarray(inputs["b_out"], dtype=np.float32)
    gn_scale = np.asarray(inputs["gn_scale"], dtype=np.float32)
    gn_bias = np.asarray(inputs["gn_bias"], dtype=np.float32)

    fmask = np.zeros((CT, P, G), dtype=np.float32)
    for t in range(CT):
        for p in range(P):
            fmask[t, p, (t * P + p) // GSZ] = 1.0
    bmask = np.ascontiguousarray(fmask.transpose(0, 2, 1))

    common = {
        "wq_t": np.ascontiguousarray(w_qkv[0:C].T),
        "wk_t": np.ascontiguousarray(w_qkv[C:2 * C].T),
        "wv_t": np.ascontiguousarray(w_qkv[2 * C:3 * C].T),
        "wo_t": np.ascontiguousarray(w_out.T),
        "b_q": np.ascontiguousarray(b_qkv[0:C].reshape(CT, P, 1)),
        "b_k": np.ascontiguousarray(b_qkv[C:2 * C].reshape(CT, P, 1)),
        "b_o": np.ascontiguousarray((b_out + w_out @ b_qkv[2 * C:3 * C]).reshape(CT, P, 1)),
        "gn_w": np.ascontiguousarray(gn_scale.reshape(CT, P, 1)),
        "gn_b": np.ascontiguousarray(gn_bias.reshape(CT, P, 1)),
        "fmask": fmask,
        "bmask": bmask,
    }
    return [dict(common, x=np.ascontiguousarray(x[b].reshape(C, N)))
            for b in range(B)]


def run(inputs, trace=False):
    nc = _get_program()
    in_maps = make_in_maps(inputs)
    res = bass_utils.run_bass_kernel_spmd(nc, in_maps, core_ids=list(range(B)),
                                          trace=trace)
    out = np.stack([res.results[b]["out"] for b in range(B)])
    return out.reshape(B, C, HH, WW), res


def kernel(**inputs):
    out, _ = run(inputs, trace=False)
    return out


# revision 6
# speedup vs baseline: 1.1795x; 1.1795x over previous
"""AttentionBlock (GroupNorm + single-head NxN attention + residual) on 8 TRN2 cores.

Data-parallel: batch dim (B=8) sharded 1 batch-image per NeuronCore. Each core
runs the full block for its image:

  x (C=256, N=4096) -> GroupNorm(8 groups) -> qkv = W_qkv @ xn (fp32r) ->
  q,k,v cast to fp8e4m3; attention runs entirely in fp8 DoubleRow matmuls
  (contraction 256 = full C or an m-pair of 128x2 keys in ONE matmul at
  2 MACs/cell/cycle):
    sT = k^T q  (one DR matmul per key-tile), e = exp(sT/16 - 4)  -> fp8
    attnout_u = v @ e (DR, m-pairs), den = ones @ e (DR)
    proj_u = W_out @ attnout_u (fp32r); out = proj_u * (1/den) + b_out + x

The softmax is unnormalized with a constant exponent shift of -4 (keeps
exp() under the fp8e4m3 max of 240; the shift and the 1/den normalization
both commute through the output projection and cancel).

Schedule: the head is pipelined (x DMA chunks -> stats -> xn slices -> k/q
projection per 512-col slice); exp is done once per key-tile PAIR over a
2-bank PSUM tile (scalar engine is the body co-bottleneck); v-tile pairs are
produced inside nch 0's attention stream, sharing a PSUM bank with the
output projection.
"""

import sys

if "/opt/trn_rl_repo" not in sys.path:
    sys.path.insert(0, "/opt/trn_rl_repo")

import numpy as np

import concourse.bass as bass
import concourse.bacc as bacc
import concourse.tile as tile
import concourse.mybir as mybir
from concourse import bass_utils

# Problem dims (hardcoded per spec)
B, C, HH, WW = 8, 256, 64, 64
N = HH * WW            # 4096
G = 8                  # groupnorm groups
GSZ = C // G           # 32 channels/group
EPS = 1e-5
P = 128                # SBUF partitions
CT = C // P            # 2 channel tiles
NCH = 512              # query-chunk width (free dim per matmul)
NNCH = N // NCH        # 8
MT = N // P            # 32 key tiles
MP = MT // 2           # 16 key-tile pairs
SCALE = 1.0 / np.sqrt(C)
ESHIFT = -4.0          # exponent shift; cancels in normalization
INV_CNT = 1.0 / (GSZ * N)

F32 = mybir.dt.float32
F32R = mybir.dt.float32r
FP8 = mybir.dt.float8e4
DR = mybir.MatmulPerfMode.DoubleRow


def _emit(tc, d, out_d):
    from contextlib import ExitStack

    nc = tc.nc
    AF = mybir.ActivationFunctionType
    OP = mybir.AluOpType
    AX = mybir.AxisListType.X
    ts, ds = bass.ts, bass.ds

    with ExitStack() as ctx:
        const = ctx.enter_context(tc.tile_pool(name="const", bufs=1))
        big = ctx.enter_context(tc.tile_pool(name="big", bufs=1))
        work = ctx.enter_context(tc.tile_pool(name="work", bufs=3))
        small = ctx.enter_context(tc.tile_pool(name="small", bufs=4))
        outp = ctx.enter_context(tc.tile_pool(name="outp", bufs=3))
        # PSUM: 8 banks total. s-pair 2 bufs x 2 banks, attn 2 banks,
        # den 1 bank, v/proj shared 1 bank.
        psS = ctx.enter_context(tc.tile_pool(name="psS", bufs=2, space="PSUM"))
        psA = ctx.enter_context(tc.tile_pool(name="psA", bufs=1, space="PSUM"))
        psD = ctx.enter_context(tc.tile_pool(name="psD", bufs=1, space="PSUM"))
        psP = ctx.enter_context(tc.tile_pool(name="psP", bufs=1, space="PSUM"))

        # ---------------- load x first (chunked; stats pipeline behind DMA) --
        NC4 = 4                      # head chunks per channel-tile
        CW = N // NC4                # 1024 columns per chunk
        x_d = d["x"]
        x_sb = big.tile([P, CT, N], F32, name="x_sb")
        for t in range(CT):
            for c in range(NC4):
                eng = nc.sync if (t * NC4 + c) % 2 == 0 else nc.gpsimd
                eng.dma_start(out=x_sb[:, t, ds(c * CW, CW)],
                              in_=x_d[ts(t, P), ds(c * CW, CW)])

        # ---------------- constants / weights to SBUF ----------------
        wq_sb = const.tile([P, CT, C], F32R, name="wq_sb")
        wk_sb = const.tile([P, CT, C], F32R, name="wk_sb")
        wv_sb = const.tile([P, CT, C], F32R, name="wv_sb")
        wo_sb = const.tile([P, CT, C], F32R, name="wo_sb")
        for sb, dr in ((wq_sb, d["wq_t"]), (wk_sb, d["wk_t"]),
                       (wv_sb, d["wv_t"]), (wo_sb, d["wo_t"])):
            for ch in range(CT):
                nc.sync.dma_start(out=sb[:, ch, :], in_=dr[ts(ch, P), :])

        bq_sb = const.tile([P, CT], F32, name="bq_sb")
        bk_sb = const.tile([P, CT], F32, name="bk_sb")
        bo_sb = const.tile([P, CT], F32, name="bo_sb")
        gw_sb = const.tile([P, CT], F32, name="gw_sb")
        gb_sb = const.tile([P, CT], F32, name="gb_sb")
        for sb, dr in ((bq_sb, d["b_q"]), (bk_sb, d["b_k"]), (bo_sb, d["b_o"]),
                       (gw_sb, d["gn_w"]), (gb_sb, d["gn_b"])):
            for t in range(CT):
                nc.sync.dma_start(out=sb[:, t:t + 1], in_=dr[t])

        fm_sb = const.tile([P, CT, G], F32, name="fm_sb")
        bm_sb = const.tile([G, CT, P], F32, name="bm_sb")
        for t in range(CT):
            nc.sync.dma_start(out=fm_sb[:, t, :], in_=d["fmask"][t])
            nc.sync.dma_start(out=bm_sb[:, t, :], in_=d["bmask"][t])

        # fp8 ones pair for the den matmul (pair-dim stride kept at 16B)
        ones_sb = const.tile([P, CT, 16], FP8, name="ones_sb")
        nc.vector.memset(ones_sb, 1.0)
        zero_sb = const.tile([P, 1], F32, name="zero_sb")
        nc.vector.memset(zero_sb, 0.0)
        esh_sb = const.tile([P, 1], F32, name="esh_sb")
        nc.vector.memset(esh_sb, ESHIFT)
        eps_sb = const.tile([G, 1], F32, name="eps_sb")
        nc.vector.memset(eps_sb, EPS)

        # ---------------- GroupNorm stats (pipelined behind x DMA) ----------
        xn_sb = big.tile([P, CT, N], F32R, name="xn_sb")
        stat = small.tile([P, CT, NC4, 2], F32, name="stat")
        for t in range(CT):
            for c in range(NC4):
                csl = ds(c * CW, CW)
                nc.vector.reduce_sum(out=stat[:, t, c, 0:1], in_=x_sb[:, t, csl],
                                     axis=AX)
                # x^2 into xn (scratch; overwritten below), row-sum into stat
                nc.scalar.activation(out=xn_sb[:, t, csl], in_=x_sb[:, t, csl],
                                     func=AF.Square, bias=zero_sb,
                                     accum_out=stat[:, t, c, 1:2])
                # PE warm-up during the head: short fp32 matmul on the chunk
                warm = psS.tile([1, P], F32, tag="s", name="warm")
                nc.tensor.matmul(warm, lhsT=zero_sb,
                                 rhs=x_sb[:, t, ds(c * CW, P)],
                                 start=True, stop=True)

        gps = psS.tile([G, 2], F32, tag="s", name="gps")
        first = True
        for t in range(CT):
            for c in range(NC4):
                nc.tensor.matmul(gps, lhsT=fm_sb[:, t, :], rhs=stat[:, t, c, :],
                                 start=first, stop=(t == CT - 1 and c == NC4 - 1))
                first = False
        grp = small.tile([G, 2], F32, name="grp")    # [mean, rstd]
        gtmp = small.tile([G, 3], F32, name="gtmp")
        nc.vector.tensor_scalar_mul(out=grp[:, 0:1], in0=gps[:, 0:1], scalar1=INV_CNT)
        nc.vector.tensor_scalar_mul(out=gtmp[:, 0:1], in0=gps[:, 1:2], scalar1=INV_CNT)
        nc.vector.tensor_mul(out=gtmp[:, 1:2], in0=grp[:, 0:1], in1=grp[:, 0:1])
        nc.vector.tensor_sub(out=gtmp[:, 2:3], in0=gtmp[:, 0:1], in1=gtmp[:, 1:2])
        nc.scalar.activation(out=gtmp[:, 2:3], in_=gtmp[:, 2:3], func=AF.Sqrt,
                             bias=eps_sb)
        nc.vector.reciprocal(out=grp[:, 1:2], in_=gtmp[:, 2:3])

        ab = small.tile([P, CT, 2], F32, name="ab")  # per-channel scale a, bias b
        for t in range(CT):
            cps = psS.tile([P, 2], F32, tag="s", name="cps")
            nc.tensor.matmul(cps, lhsT=bm_sb[:, t, :], rhs=grp, start=True, stop=True)
            nc.vector.tensor_mul(out=ab[:, t, 0:1], in0=cps[:, 1:2], in1=gw_sb[:, t:t + 1])
            nc.vector.tensor_mul(out=ab[:, t, 1:2], in0=cps[:, 0:1], in1=ab[:, t, 0:1])
            nc.vector.tensor_sub(out=ab[:, t, 1:2], in0=gb_sb[:, t:t + 1], in1=ab[:, t, 1:2])

        # ---------------- xn slices + K/Q projections, pipelined ------------
        # Per 512-col slice: normalize both channel halves, then K (and later
        # Q) projections of that slice. fp32r matmul -> bias-add -> fp8 store.
        q_sb = big.tile([P, CT, N], FP8, name="q_sb")   # (c_half, n)
        k_sb = big.tile([P, CT, N], FP8, name="k_sb")
        vT_sb = big.tile([P, MT, C], FP8, name="vT_sb")  # (n, c), n on partitions

        for s in range(NNCH):
            nsl = ds(s * NCH, NCH)
            for t in range(CT):
                eng = nc.vector if (s * CT + t) % 2 == 0 else nc.gpsimd
                eng.tensor_scalar(out=xn_sb[:, t, nsl], in0=x_sb[:, t, nsl],
                                  scalar1=ab[:, t, 0:1], scalar2=ab[:, t, 1:2],
                                  op0=OP.mult, op1=OP.add)

        qki = 0
        for (w_sb, b_sb, o_sb) in ((wk_sb, bk_sb, k_sb), (wq_sb, bq_sb, q_sb)):
            for s in range(NNCH):
                nsl = ds(s * NCH, NCH)
                pp = psS.tile([P, CT, NCH], F32, tag="s", name="ppqk")
                for tq in range(CT):
                    for ch in range(CT):
                        nc.tensor.matmul(
                            pp[:, tq, :],
                            lhsT=w_sb[:, ch, ts(tq, P)],
                            rhs=xn_sb[:, ch, nsl],
                            start=(ch == 0), stop=(ch == CT - 1))
                for tq in range(CT):
                    osl = o_sb[:, tq, nsl]
                    if qki % 2 == 0:
                        nc.vector.tensor_scalar_add(out=osl, in0=pp[:, tq, :],
                                                    scalar1=b_sb[:, tq:tq + 1])
                    else:
                        nc.scalar.activation(out=osl, in_=pp[:, tq, :],
                                             func=AF.Identity,
                                             bias=b_sb[:, tq:tq + 1])
                    qki += 1

        # ---------------- attention + output projection (fp8 DoubleRow) -----
        # v-tile pairs are produced inside nch 0's stream (PSUM bank shared
        # with proj; b_v is folded into b_out host-side).
        for nch in range(NNCH):
            nsl = ds(nch * NCH, NCH)
            attn = psA.tile([P, CT, NCH], F32, tag="attn", name="attn")
            den = psD.tile([1, NCH], F32, tag="den", name="den")
            for mp in range(MP):
                if nch == 0:
                    vt = psP.tile([P, CT, C], F32, tag="proj", name="vt")
                    for half in range(2):
                        for ch in range(CT):
                            nc.tensor.matmul(vt[:, half, :],
                                             lhsT=xn_sb[:, ch, ts(2 * mp + half, P)],
                                             rhs=wv_sb[:, ch, :],
                                             start=(ch == 0), stop=(ch == CT - 1))
                    nc.vector.tensor_copy(out=vT_sb[:, ds(2 * mp, 2), :], in_=vt)
                sp = psS.tile([P, CT, NCH], F32, tag="s", name="sp")
                for half in range(2):
                    nc.tensor.matmul(sp[:, half, :],
                                     lhsT=k_sb[:, :, ts(2 * mp + half, P)],
                                     rhs=q_sb[:, :, nsl],
                                     start=True, stop=True, perf_mode=DR)
                e = work.tile([P, 2, NCH], FP8, tag="e", name="e", bufs=4)
                nc.scalar.activation(out=e, in_=sp, func=AF.Exp,
                                     bias=esh_sb, scale=SCALE)
                for ch in range(CT):
                    nc.tensor.matmul(attn[:, ch, :],
                                     lhsT=vT_sb[:, ds(2 * mp, 2), ts(ch, P)],
                                     rhs=e,
                                     start=(mp == 0), stop=(mp == MP - 1),
                                     perf_mode=DR)
                nc.tensor.matmul(den, lhsT=ones_sb[:, :, 0:1], rhs=e,
                                 start=(mp == 0), stop=(mp == MP - 1),
                                 perf_mode=DR)

            den_sb = small.tile([1, NCH], F32, tag="den_sb", name="den_sb", bufs=2)
            nc.vector.tensor_copy(out=den_sb, in_=den)
            rden = small.tile([1, NCH], F32, tag="rden", name="rden", bufs=2)
            rscr = small.tile([1, NCH], F32, tag="rscr", name="rscr", bufs=2)
            nc.vector.reciprocal_approx_accurate(out=rden, in_=den_sb, scratch=rscr)
            rdenb = outp.tile([P, NCH], F32, tag="rdenb", name="rdenb", bufs=2)
            nc.gpsimd.partition_broadcast(rdenb, rden)

            atts = outp.tile([P, CT, NCH], F32R, tag="att", name="atts", bufs=2)
            nc.vector.tensor_copy(out=atts, in_=attn)

            # proj per output-channel half on a single PSUM bank; copy to SBUF
            # immediately (PSUM release must not be gated on the rden chain)
            for co in range(CT):
                pj = psP.tile([P, NCH], F32, tag="proj", name="pj")
                for ch in range(CT):
                    nc.tensor.matmul(pj,
                                     lhsT=wo_sb[:, ch, ts(co, P)],
                                     rhs=atts[:, ch, :],
                                     start=(ch == 0), stop=(ch == CT - 1))
                pjs = outp.tile([P, NCH], F32, tag="pjs", name="pjs", bufs=2)
                nc.vector.tensor_copy(out=pjs, in_=pj)
                f = outp.tile([P, NCH], F32, tag="fout", name="f", bufs=2)
                nc.vector.tensor_tensor(out=f, in0=pjs, in1=rdenb, op=OP.mult)
                nc.vector.scalar_tensor_tensor(out=f, in0=f, scalar=bo_sb[:, co:co + 1],
                                               in1=x_sb[:, co, nsl],
                                               op0=OP.add, op1=OP.add)
                nc.sync.dma_start(out=out_d[ts(co, P), nsl], in_=f)


def build_program():
    nc = bacc.Bacc("TRN2", target_bir_lowering=False, debug=False, num_devices=B)
    d = {}

    def din(name, shape, dt_=F32):
        d[name] = nc.dram_tensor(name, list(shape), dt_, kind="ExternalInput").ap()

    din("x", (C, N))
    din("wq_t", (C, C), F32R)
    din("wk_t", (C, C), F32R)
    din("wv_t", (C, C), F32R)
    din("wo_t", (C, C), F32R)
    din("b_q", (CT, P, 1))
    din("b_k", (CT, P, 1))
    din("b_o", (CT, P, 1))
    din("gn_w", (CT, P, 1))
    din("gn_b", (CT, P, 1))
    din("fmask", (CT, P, G))
    din("bmask", (CT, G, P))
    out_d = nc.dram_tensor("out", [C, N], F32, kind="ExternalOutput").ap()

    with tile.TileContext(nc) as tc:
        _emit(tc, d, out_d)
    nc.compile()
    return nc


_PROG = None


def _get_program():
    global _PROG
    if _PROG is None:
        _PROG = build_program()
    return _PROG


def make_in_maps(inputs):
    x = np.ascontiguousarray(np.asarray(inputs["x"], dtype=np.float32))
    w_qkv = np.asarray(inputs["w_qkv"], dtype=np.float32)
    b_qkv = np.asarray(inputs["b_qkv"], dtype=np.float32)
    w_out = np.asarray(inputs["w_out"], dtype=np.float32)
    b_out = np.asarray(inputs["b_out"], dtype=np.float32)
    gn_scale = np.asarray(inputs["gn_scale"], dtype=np.float32)
    gn_bias = np.asarray(inputs["gn_bias"], dtype=np.float32)

    fmask = np.zeros((CT, P, G), dtype=np.float32)
    for t in range(CT):
        for p in range(P):
            fmask[t, p, (t * P + p) // GSZ] = 1.0
    bmask = np.ascontiguousarray(fmask.transpose(0, 2, 1))

    common = {
        "wq_t": np.ascontiguousarray(w_qkv[0:C].T),
        "wk_t": np.ascontiguousarray(w_qkv[C:2 * C].T),
        "wv_t": np.ascontiguousarray(w_qkv[2 * C:3 * C].T),
        "wo_t": np.ascontiguousarray(w_out.T),
        "b_q": np.ascontiguousarray(b_qkv[0:C].reshape(CT, P, 1)),
        "b_k": np.ascontiguousarray(b_qkv[C:2 * C].reshape(CT, P, 1)),
        "b_o": np.ascontiguousarray((b_out + w_out @ b_qkv[2 * C:3 * C]).reshape(CT, P, 1)),
        "gn_w": np.ascontiguousarray(gn_scale.reshape(CT, P, 1)),
        "gn_b": np.ascontiguousarray(gn_bias.reshape(CT, P, 1)),
        "fmask": fmask,
        "bmask": bmask,
    }
    return [dict(common, x=np.ascontiguousarray(x[b].reshape(C, N)))
            for b in range(B)]


def run(inputs, trace=False):
    nc = _get_program()
    in_maps = make_in_maps(inputs)
    res = bass_utils.run_bass_kernel_spmd(nc, in_maps, core_ids=list(range(B)),
                                          trace=trace)
    out = np.stack([res.results[b]["out"] for b in range(B)])
    return out.reshape(B, C, HH, WW), res


def kernel(**inputs):
    out, _ = run(inputs, trace=False)
    return out


# revision 15
# speedup vs baseline: 1.2098x; 1.0256x over previous
"""AttentionBlock (GroupNorm + single-head NxN attention + residual) on 8 TRN2 cores.

Data-parallel: batch dim (B=8) sharded 1 batch-image per NeuronCore. Each core
runs the full block for its image:

  x (C=256, N=4096) -> GroupNorm(8 groups) -> qkv = W_qkv @ xn (fp32r) ->
  q,k,v cast to fp8e4m3; attention runs entirely in fp8 DoubleRow matmuls
  (contraction 256 = full C or an m-pair of 128x2 keys in ONE matmul at
  2 MACs/cell/cycle):
    sT = k^T q  (one DR matmul per key-tile), e = exp(sT/16 - 4)  -> fp8
    attnout_u = v @ e (DR, m-pairs), den = ones @ e (DR)
    proj_u = W_out @ attnout_u (fp32r); out = proj_u * (1/den) + b_out + x

The softmax is unnormalized with a constant exponent shift of -4 (keeps
exp() under the fp8e4m3 max of 240; the shift and the 1/den normalization
both commute through the output projection and cancel).

Schedule: the head is pipelined (x DMA chunks -> stats -> xn slices -> k/q
projection per 512-col slice); exp is done once per key-tile PAIR over a
2-bank PSUM tile (scalar engine is the body co-bottleneck); v-tile pairs are
produced inside nch 0's attention stream, sharing a PSUM bank with the
output projection.
"""

import sys

if "/opt/trn_rl_repo" not in sys.path:
    sys.path.insert(0, "/opt/trn_rl_repo")

import numpy as np

import concourse.bass as bass
import concourse.bacc as bacc
import concourse.tile as tile
import concourse.mybir as mybir
from concourse import bass_utils

# Problem dims (hardcoded per spec)
B, C, HH, WW = 8, 256, 64, 64
N = HH * WW            # 4096
G = 8                  # groupnorm groups
GSZ = C // G           # 32 channels/group
EPS = 1e-5
P = 128                # SBUF partitions
CT = C // P            # 2 channel tiles
NCH = 512              # query-chunk width (free dim per matmul)
NNCH = N // NCH        # 8
MT = N // P            # 32 key tiles
MP = MT // 2           # 16 key-tile pairs
SCALE = 1.0 / np.sqrt(C)
ESHIFT = -4.0          # exponent shift; cancels in normalization
INV_CNT = 1.0 / (GSZ * N)

F32 = mybir.dt.float32
F32R = mybir.dt.float32r
FP8 = mybir.dt.float8e4
DR = mybir.MatmulPerfMode.DoubleRow


def _emit(tc, d, out_d):
    from contextlib import ExitStack

    nc = tc.nc
    AF = mybir.ActivationFunctionType
    OP = mybir.AluOpType
    AX = mybir.AxisListType.X
    ts, ds = bass.ts, bass.ds

    with ExitStack() as ctx:
        const = ctx.enter_context(tc.tile_pool(name="const", bufs=1))
        big = ctx.enter_context(tc.tile_pool(name="big", bufs=1))
        work = ctx.enter_context(tc.tile_pool(name="work", bufs=3))
        small = ctx.enter_context(tc.tile_pool(name="small", bufs=4))
        outp = ctx.enter_context(tc.tile_pool(name="outp", bufs=3))
        # PSUM: 8 banks total. s-pair 2 bufs x 2 banks, attn 2 banks,
        # den 1 bank, v/proj shared 1 bank.
        psS = ctx.enter_context(tc.tile_pool(name="psS", bufs=2, space="PSUM"))
        psA = ctx.enter_context(tc.tile_pool(name="psA", bufs=1, space="PSUM"))
        psD = ctx.enter_context(tc.tile_pool(name="psD", bufs=1, space="PSUM"))
        psP = ctx.enter_context(tc.tile_pool(name="psP", bufs=1, space="PSUM"))

        # ---------------- DMAs: packed consts first, then x on 4 queues -----
        # consts_a columns: b_q(2) b_k(2) b_o(2) gn_w(2) gn_b(2) fmask(2x8)
        ca = const.tile([P, 26], F32, name="ca")
        nc.scalar.dma_start(out=ca, in_=d["consts_a"])
        bm_sb = const.tile([G, CT, P], F32, name="bm_sb")
        nc.scalar.dma_start(out=bm_sb[:, :, :], in_=d["bmask"])
        BQ, BK, BO = 0, 2, 4         # ca column offsets

        NC4 = 4                      # head chunks per channel-tile
        CW = N // NC4                # 1024 columns per chunk
        x_d = d["x"]
        x_sb = big.tile([P, CT, N], F32, name="x_sb")
        xq = [nc.sync, nc.gpsimd, nc.scalar]
        for c in range(NC4):
            for t in range(CT):
                csl = ds(c * CW, CW)
                xq[(c * CT + t) % 3].dma_start(out=x_sb[:, t, csl],
                                               in_=x_d[ts(t, P), csl])

        wq_sb = const.tile([P, CT, C], F32R, name="wq_sb")
        wk_sb = const.tile([P, CT, C], F32R, name="wk_sb")
        wv_sb = const.tile([P, CT, C], F32R, name="wv_sb")
        wo_sb = const.tile([P, CT, C], F32R, name="wo_sb")
        for i, (sb, dr) in enumerate(((wk_sb, d["wk_t"]), (wq_sb, d["wq_t"]),
                                      (wv_sb, d["wv_t"]), (wo_sb, d["wo_t"]))):
            for ch in range(CT):
                eng = nc.sync if i % 2 == 0 else nc.gpsimd
                eng.dma_start(out=sb[:, ch, :], in_=dr[ts(ch, P), :])

        # fp8 ones pair for the den matmul (pair-dim stride kept at 16B)
        ones_sb = const.tile([P, CT, 16], FP8, name="ones_sb")
        nc.gpsimd.memset(ones_sb, 1.0)
        zero_sb = const.tile([P, 1], F32, name="zero_sb")
        nc.gpsimd.memset(zero_sb, 0.0)
        esh_sb = const.tile([P, 1], F32, name="esh_sb")
        nc.gpsimd.memset(esh_sb, ESHIFT)
        eps_sb = const.tile([G, 1], F32, name="eps_sb")
        nc.gpsimd.memset(eps_sb, EPS)

        # ---------------- GroupNorm stats (pipelined behind x DMA) ----------
        # Per chunk: row-sums on vector (t0) / gpsimd (t1), x^2 row-sums on
        # scalar; group-combine matmul accumulates as chunks complete.
        xn_sb = big.tile([P, CT, N], F32R, name="xn_sb")
        stat = small.tile([P, CT, NC4, 2], F32, name="stat")
        gps = psS.tile([G, 2], F32, tag="s", name="gps")
        for c in range(NC4):
            for t in range(CT):
                csl = ds(c * CW, CW)
                nc.vector.reduce_sum(out=stat[:, t, c, 0:1], in_=x_sb[:, t, csl],
                                     axis=AX)
                # x^2 into xn (scratch; overwritten below), row-sum into stat
                nc.scalar.activation(out=xn_sb[:, t, csl], in_=x_sb[:, t, csl],
                                     func=AF.Square, bias=zero_sb,
                                     accum_out=stat[:, t, c, 1:2])
                # PE warm-up during the head: short fp32 matmul on the chunk
                warm = psS.tile([1, P], F32, tag="s", name="warm")
                nc.tensor.matmul(warm, lhsT=zero_sb,
                                 rhs=x_sb[:, t, ds(c * CW, P)],
                                 start=True, stop=True)
            for t in range(CT):
                nc.tensor.matmul(gps, lhsT=ca[:, ds(10 + G * t, G)],
                                 rhs=stat[:, t, c, :],
                                 start=(c == 0 and t == 0),
                                 stop=(c == NC4 - 1 and t == CT - 1))
        grp = small.tile([G, 2], F32, name="grp")    # [mean, rstd]
        gtmp = small.tile([G, 3], F32, name="gtmp")
        nc.vector.tensor_scalar_mul(out=grp[:, 0:1], in0=gps[:, 0:1], scalar1=INV_CNT)
        nc.vector.tensor_scalar_mul(out=gtmp[:, 0:1], in0=gps[:, 1:2], scalar1=INV_CNT)
        nc.vector.tensor_mul(out=gtmp[:, 1:2], in0=grp[:, 0:1], in1=grp[:, 0:1])
        nc.vector.tensor_sub(out=gtmp[:, 2:3], in0=gtmp[:, 0:1], in1=gtmp[:, 1:2])
        nc.scalar.activation(out=gtmp[:, 2:3], in_=gtmp[:, 2:3], func=AF.Sqrt,
                             bias=eps_sb)
        nc.vector.reciprocal(out=grp[:, 1:2], in_=gtmp[:, 2:3])

        ab = small.tile([P, CT, 2], F32, name="ab")  # per-channel scale a, bias b
        for t in range(CT):
            cps = psS.tile([P, 2], F32, tag="s", name="cps")
            nc.tensor.matmul(cps, lhsT=bm_sb[:, t, :], rhs=grp, start=True, stop=True)
            nc.vector.tensor_mul(out=ab[:, t, 0:1], in0=cps[:, 1:2], in1=ca[:, 6 + t:7 + t])
            nc.vector.tensor_mul(out=ab[:, t, 1:2], in0=cps[:, 0:1], in1=ab[:, t, 0:1])
            nc.vector.tensor_sub(out=ab[:, t, 1:2], in0=ca[:, 8 + t:9 + t], in1=ab[:, t, 1:2])
            # keep the PE ticking until the K/Q matmuls start
            warm = psS.tile([1, P], F32, tag="s", name="warm2")
            nc.tensor.matmul(warm, lhsT=zero_sb, rhs=x_sb[:, t, 0:P],
                             start=True, stop=True)

        # ---------------- xn slices + K/Q projections, pipelined ------------
        # Per 512-col slice: normalize both channel halves, then K (and later
        # Q) projections of that slice. fp32r matmul -> bias-add -> fp8 store.
        q_sb = big.tile([P, CT, N], FP8, name="q_sb")   # (c_half, n)
        k_sb = big.tile([P, CT, N], FP8, name="k_sb")
        vT_sb = big.tile([P, MT, C], FP8, name="vT_sb")  # (n, c), n on partitions

        for s in range(NNCH):
            nsl = ds(s * NCH, NCH)
            for t in range(CT):
                eng = nc.vector if (s * CT + t) % 2 == 0 else nc.gpsimd
                eng.tensor_scalar(out=xn_sb[:, t, nsl], in0=x_sb[:, t, nsl],
                                  scalar1=ab[:, t, 0:1], scalar2=ab[:, t, 1:2],
                                  op0=OP.mult, op1=OP.add)

        qki = 0
        for (w_sb, bcol, o_sb) in ((wk_sb, BK, k_sb), (wq_sb, BQ, q_sb)):
            for s in range(NNCH):
                nsl = ds(s * NCH, NCH)
                pp = psS.tile([P, CT, NCH], F32, tag="s", name="ppqk")
                for tq in range(CT):
                    for ch in range(CT):
                        nc.tensor.matmul(
                            pp[:, tq, :],
                            lhsT=w_sb[:, ch, ts(tq, P)],
                            rhs=xn_sb[:, ch, nsl],
                            start=(ch == 0), stop=(ch == CT - 1))
                for tq in range(CT):
                    osl = o_sb[:, tq, nsl]
                    if qki % 2 == 0:
                        nc.vector.tensor_scalar_add(out=osl, in0=pp[:, tq, :],
                                                    scalar1=ca[:, bcol + tq:bcol + tq + 1])
                    else:
                        nc.scalar.activation(out=osl, in_=pp[:, tq, :],
                                             func=AF.Identity,
                                             bias=ca[:, bcol + tq:bcol + tq + 1])
                    qki += 1

        # ---------------- attention + output projection (fp8 DoubleRow) -----
        # v-tile pairs are produced inside nch 0's stream (PSUM bank shared
        # with proj; b_v is folded into b_out host-side).
        for nch in range(NNCH):
            nsl = ds(nch * NCH, NCH)
            attn = psA.tile([P, CT, NCH], F32, tag="attn", name="attn")
            den = psD.tile([1, NCH], F32, tag="den", name="den")
            for mp in range(MP):
                if nch == 0:
                    vt = psP.tile([P, CT, C], F32, tag="proj", name="vt")
                    for half in range(2):
                        for ch in range(CT):
                            nc.tensor.matmul(vt[:, half, :],
                                             lhsT=xn_sb[:, ch, ts(2 * mp + half, P)],
                                             rhs=wv_sb[:, ch, :],
                                             start=(ch == 0), stop=(ch == CT - 1))
                    nc.vector.tensor_copy(out=vT_sb[:, ds(2 * mp, 2), :], in_=vt)
                sp = psS.tile([P, CT, NCH], F32, tag="s", name="sp")
                for half in range(2):
                    nc.tensor.matmul(sp[:, half, :],
                                     lhsT=k_sb[:, :, ts(2 * mp + half, P)],
                                     rhs=q_sb[:, :, nsl],
                                     start=True, stop=True, perf_mode=DR)
                e = work.tile([P, 2, NCH], FP8, tag="e", name="e", bufs=4)
                nc.scalar.activation(out=e, in_=sp, func=AF.Exp,
                                     bias=esh_sb, scale=SCALE)
                for ch in range(CT):
                    nc.tensor.matmul(attn[:, ch, :],
                                     lhsT=vT_sb[:, ds(2 * mp, 2), ts(ch, P)],
                                     rhs=e,
                                     start=(mp == 0), stop=(mp == MP - 1),
                                     perf_mode=DR)
                nc.tensor.matmul(den, lhsT=ones_sb[:, :, 0:1], rhs=e,
                                 start=(mp == 0), stop=(mp == MP - 1),
                                 perf_mode=DR)

            den_sb = small.tile([1, NCH], F32, tag="den_sb", name="den_sb", bufs=2)
            nc.vector.tensor_copy(out=den_sb, in_=den)
            rden = small.tile([1, NCH], F32, tag="rden", name="rden", bufs=2)
            rscr = small.tile([1, NCH], F32, tag="rscr", name="rscr", bufs=2)
            nc.vector.reciprocal_approx_accurate(out=rden, in_=den_sb, scratch=rscr)
            rdenb = outp.tile([P, NCH], F32, tag="rdenb", name="rdenb", bufs=2)
            nc.gpsimd.partition_broadcast(rdenb, rden)

            atts = outp.tile([P, CT, NCH], F32R, tag="att", name="atts", bufs=2)
            nc.vector.tensor_copy(out=atts, in_=attn)

            # proj per output-channel half on a single PSUM bank; copy to SBUF
            # immediately (PSUM release must not be gated on the rden chain)
            for co in range(CT):
                pj = psP.tile([P, NCH], F32, tag="proj", name="pj")
                for ch in range(CT):
                    nc.tensor.matmul(pj,
                                     lhsT=wo_sb[:, ch, ts(co, P)],
                                     rhs=atts[:, ch, :],
                                     start=(ch == 0), stop=(ch == CT - 1))
                pjs = outp.tile([P, NCH], F32, tag="pjs", name="pjs", bufs=2)
                nc.vector.tensor_copy(out=pjs, in_=pj)
                f = outp.tile([P, NCH], F32, tag="fout", name="f", bufs=2)
                nc.vector.tensor_tensor(out=f, in0=pjs, in1=rdenb, op=OP.mult)
                nc.vector.scalar_tensor_tensor(out=f, in0=f, scalar=ca[:, BO + co:BO + co + 1],
                                               in1=x_sb[:, co, nsl],
                                               op0=OP.add, op1=OP.add)
                nc.sync.dma_start(out=out_d[ts(co, P), nsl], in_=f)


def build_program():
    nc = bacc.Bacc("TRN2", target_bir_lowering=False, debug=False, num_devices=B)
    d = {}

    def din(name, shape, dt_=F32):
        d[name] = nc.dram_tensor(name, list(shape), dt_, kind="ExternalInput").ap()

    din("x", (C, N))
    din("wq_t", (C, C), F32R)
    din("wk_t", (C, C), F32R)
    din("wv_t", (C, C), F32R)
    din("wo_t", (C, C), F32R)
    din("consts_a", (P, 26))
    din("bmask", (G, CT * P))
    out_d = nc.dram_tensor("out", [C, N], F32, kind="ExternalOutput").ap()

    with tile.TileContext(nc) as tc:
        _emit(tc, d, out_d)
    nc.compile()
    return nc


_PROG = None


def _get_program():
    global _PROG
    if _PROG is None:
        _PROG = build_program()
    return _PROG


def make_in_maps(inputs):
    x = np.ascontiguousarray(np.asarray(inputs["x"], dtype=np.float32))
    w_qkv = np.asarray(inputs["w_qkv"], dtype=np.float32)
    b_qkv = np.asarray(inputs["b_qkv"], dtype=np.float32)
    w_out = np.asarray(inputs["w_out"], dtype=np.float32)
    b_out = np.asarray(inputs["b_out"], dtype=np.float32)
    gn_scale = np.asarray(inputs["gn_scale"], dtype=np.float32)
    gn_bias = np.asarray(inputs["gn_bias"], dtype=np.float32)

    fmask = np.zeros((CT, P, G), dtype=np.float32)
    for t in range(CT):
        for p in range(P):
            fmask[t, p, (t * P + p) // GSZ] = 1.0
    # bmask[g, t*P+p] = fmask[t, p, g]
    bmask = np.ascontiguousarray(fmask.transpose(2, 0, 1).reshape(G, CT * P))

    consts_a = np.zeros((P, 26), dtype=np.float32)
    bo_eff = b_out + w_out @ b_qkv[2 * C:3 * C]   # b_v folded (softmax sums to 1)
    for t in range(CT):
        rows = slice(t * P, (t + 1) * P)
        consts_a[:, 0 + t] = b_qkv[0:C][rows]
        consts_a[:, 2 + t] = b_qkv[C:2 * C][rows]
        consts_a[:, 4 + t] = bo_eff[rows]
        consts_a[:, 6 + t] = gn_scale[rows]
        consts_a[:, 8 + t] = gn_bias[rows]
        consts_a[:, 10 + G * t:10 + G * (t + 1)] = fmask[t]

    common = {
        "wq_t": np.ascontiguousarray(w_qkv[0:C].T),
        "wk_t": np.ascontiguousarray(w_qkv[C:2 * C].T),
        "wv_t": np.ascontiguousarray(w_qkv[2 * C:3 * C].T),
        "wo_t": np.ascontiguousarray(w_out.T),
        "consts_a": consts_a,
        "bmask": bmask,
    }
    return [dict(common, x=np.ascontiguousarray(x[b].reshape(C, N)))
            for b in range(B)]


def run(inputs, trace=False):
    nc = _get_program()
    in_maps = make_in_maps(inputs)
    res = bass_utils.run_bass_kernel_spmd(nc, in_maps, core_ids=list(range(B)),
                                          trace=trace)
    out = np.stack([res.results[b]["out"] for b in range(B)])
    return out.reshape(B, C, HH, WW), res


def kernel(**inputs):
    out, _ = run(inputs, trace=False)
    return out


# revision 21
# speedup vs baseline: 1.2768x; 1.0554x over previous
"""AttentionBlock (GroupNorm + single-head NxN attention + residual) on 8 TRN2 cores.

Data-parallel: batch dim (B=8) sharded 1 batch-image per NeuronCore. Each core
runs the full block for its image:

  x (C=256, N=4096) -> GroupNorm(8 groups) -> qkv = W_qkv @ xn (fp32r) ->
  q,k,v cast to fp8e4m3; attention runs entirely in fp8 DoubleRow matmuls
  (contraction 256 = full C or an m-pair of 128x2 keys in ONE matmul at
  2 MACs/cell/cycle):
    sT = k^T q  (one DR matmul per key-tile), e = exp(sT/16 - 4)  -> fp8
    attnout_u = v @ e (DR, m-pairs), den = ones @ e (DR)
    proj_u = W_out @ attnout_u (fp32r); out = proj_u * (1/den) + b_out + x

The softmax is unnormalized with a constant exponent shift of -4 (keeps
exp() under the fp8e4m3 max of 240; the shift and the 1/den normalization
both commute through the output projection and cancel).

Schedule: the head is pipelined (x DMA chunks -> stats -> xn slices -> k/q
projection per 512-col slice); exp is done once per key-tile PAIR over a
2-bank PSUM tile (scalar engine is the body co-bottleneck); v-tile pairs are
produced inside nch 0's attention stream, sharing a PSUM bank with the
output projection.
"""

import sys

if "/opt/trn_rl_repo" not in sys.path:
    sys.path.insert(0, "/opt/trn_rl_repo")

import numpy as np

import concourse.bass as bass
import concourse.bacc as bacc
import concourse.tile as tile
import concourse.mybir as mybir
from concourse import bass_utils

# Problem dims (hardcoded per spec)
B, C, HH, WW = 8, 256, 64, 64
N = HH * WW            # 4096
G = 8                  # groupnorm groups
GSZ = C // G           # 32 channels/group
EPS = 1e-5
P = 128                # SBUF partitions
CT = C // P            # 2 channel tiles
NCH = 512              # query-chunk width (free dim per matmul)
NNCH = N // NCH        # 8
MT = N // P            # 32 key tiles
MP = MT // 2           # 16 key-tile pairs
SCALE = 1.0 / np.sqrt(C)
ESHIFT = -4.0          # exponent shift; cancels in normalization
INV_CNT = 1.0 / (GSZ * N)

F32 = mybir.dt.float32
F32R = mybir.dt.float32r
FP8 = mybir.dt.float8e4
DR = mybir.MatmulPerfMode.DoubleRow


def _emit(tc, d, out_d):
    from contextlib import ExitStack

    nc = tc.nc
    AF = mybir.ActivationFunctionType
    OP = mybir.AluOpType
    AX = mybir.AxisListType.X
    ts, ds = bass.ts, bass.ds

    with ExitStack() as ctx:
        const = ctx.enter_context(tc.tile_pool(name="const", bufs=1))
        big = ctx.enter_context(tc.tile_pool(name="big", bufs=1))
        work = ctx.enter_context(tc.tile_pool(name="work", bufs=3))
        small = ctx.enter_context(tc.tile_pool(name="small", bufs=4))
        outp = ctx.enter_context(tc.tile_pool(name="outp", bufs=3))
        # PSUM: 8 banks total. s-pair 2 bufs x 2 banks, attn 2 banks,
        # den 1 bank, v/proj shared 1 bank.
        psS = ctx.enter_context(tc.tile_pool(name="psS", bufs=2, space="PSUM"))
        psA = ctx.enter_context(tc.tile_pool(name="psA", bufs=1, space="PSUM"))
        psD = ctx.enter_context(tc.tile_pool(name="psD", bufs=1, space="PSUM"))
        psP = ctx.enter_context(tc.tile_pool(name="psP", bufs=1, space="PSUM"))

        # ---------------- DMAs: packed consts first, then x on 4 queues -----
        # consts_a columns: b_q(2) b_k(2) b_o(2) gn_w(2) gn_b(2) fmask(2x8)
        ca = const.tile([P, 26], F32, name="ca")
        nc.scalar.dma_start(out=ca, in_=d["consts_a"])
        bm_sb = const.tile([G, CT, P], F32, name="bm_sb")
        nc.scalar.dma_start(out=bm_sb[:, :, :], in_=d["bmask"])
        BQ, BK, BO = 0, 2, 4         # ca column offsets

        NC4 = 8                      # head chunks per channel-tile
        CW = N // NC4                # 512 columns per chunk
        x_d = d["x"]
        x_sb = big.tile([P, CT, N], F32, name="x_sb")
        xq = [nc.sync, nc.gpsimd, nc.scalar]
        for c in range(NC4):
            for t in range(CT):
                csl = ds(c * CW, CW)
                xq[(c * CT + t) % 3].dma_start(out=x_sb[:, t, csl],
                                               in_=x_d[ts(t, P), csl])

        # fp8 pair-packed qkv weights ([cin_half, 2, cout]); wo stays f32r
        wq_sb = const.tile([P, CT, C], FP8, name="wq_sb")
        wk_sb = const.tile([P, CT, C], FP8, name="wk_sb")
        wv_sb = const.tile([P, CT, C], FP8, name="wv_sb")
        wo_sb = const.tile([P, CT, C], F32R, name="wo_sb")
        for i, (sb, dr) in enumerate(((wk_sb, d["wk8"]), (wq_sb, d["wq8"]),
                                      (wv_sb, d["wv8"]))):
            xq[i % 3].dma_start(out=sb, in_=dr)
        for ch in range(CT):
            nc.sync.dma_start(out=wo_sb[:, ch, :], in_=d["wo_t"][ts(ch, P), :])

        # fp8 ones pair for the den matmul (pair-dim stride kept at 16B)
        ones_sb = const.tile([P, CT, 16], FP8, name="ones_sb")
        nc.gpsimd.memset(ones_sb, 1.0)
        zero_sb = const.tile([P, 1], F32, name="zero_sb")
        nc.gpsimd.memset(zero_sb, 0.0)
        esh_sb = const.tile([P, 1], F32, name="esh_sb")
        nc.gpsimd.memset(esh_sb, ESHIFT)
        eps_sb = const.tile([G, 1], F32, name="eps_sb")
        nc.gpsimd.memset(eps_sb, EPS)

        # ---------------- GroupNorm stats (pipelined behind x DMA) ----------
        # Per chunk: row-sums on vector, x^2 row-sums on scalar (accum_out;
        # the squared values land in a throwaway f32 scratch); group-combine
        # matmul accumulates as chunks complete.
        xn_sb = big.tile([P, CT, N], FP8, name="xn_sb")
        sq_scr = small.tile([P, 2, CW], F32, name="sq_scr")
        stat = small.tile([P, CT, NC4, 2], F32, name="stat")
        gps = psS.tile([G, 2], F32, tag="s", name="gps")
        for c in range(NC4):
            for t in range(CT):
                csl = ds(c * CW, CW)
                nc.vector.reduce_sum(out=stat[:, t, c, 0:1], in_=x_sb[:, t, csl],
                                     axis=AX)
                nc.scalar.activation(out=sq_scr[:, (c * CT + t) % 2, :],
                                     in_=x_sb[:, t, csl],
                                     func=AF.Square, bias=zero_sb,
                                     accum_out=stat[:, t, c, 1:2])
                # PE warm-up during the head: short fp32 matmul on the chunk
                if t == 0:
                    warm = psS.tile([1, P], F32, tag="s", name="warm")
                    nc.tensor.matmul(warm, lhsT=zero_sb,
                                     rhs=x_sb[:, t, ds(c * CW, P)],
                                     start=True, stop=True)
            for t in range(CT):
                nc.tensor.matmul(gps, lhsT=ca[:, ds(10 + G * t, G)],
                                 rhs=stat[:, t, c, :],
                                 start=(c == 0 and t == 0),
                                 stop=(c == NC4 - 1 and t == CT - 1))
        grp = small.tile([G, 2], F32, name="grp")    # [mean, rstd]
        gtmp = small.tile([G, 3], F32, name="gtmp")
        nc.vector.tensor_scalar_mul(out=grp[:, 0:1], in0=gps[:, 0:1], scalar1=INV_CNT)
        nc.vector.tensor_scalar_mul(out=gtmp[:, 0:1], in0=gps[:, 1:2], scalar1=INV_CNT)
        nc.vector.tensor_mul(out=gtmp[:, 1:2], in0=grp[:, 0:1], in1=grp[:, 0:1])
        nc.vector.tensor_sub(out=gtmp[:, 2:3], in0=gtmp[:, 0:1], in1=gtmp[:, 1:2])
        nc.scalar.activation(out=gtmp[:, 2:3], in_=gtmp[:, 2:3], func=AF.Sqrt,
                             bias=eps_sb)
        nc.vector.reciprocal(out=grp[:, 1:2], in_=gtmp[:, 2:3])

        ab = small.tile([P, CT, 2], F32, name="ab")  # per-channel scale a, bias b
        for t in range(CT):
            cps = psS.tile([P, 2], F32, tag="s", name="cps")
            nc.tensor.matmul(cps, lhsT=bm_sb[:, t, :], rhs=grp, start=True, stop=True)
            nc.vector.tensor_mul(out=ab[:, t, 0:1], in0=cps[:, 1:2], in1=ca[:, 6 + t:7 + t])
            nc.vector.tensor_mul(out=ab[:, t, 1:2], in0=cps[:, 0:1], in1=ab[:, t, 0:1])
            nc.vector.tensor_sub(out=ab[:, t, 1:2], in0=ca[:, 8 + t:9 + t], in1=ab[:, t, 1:2])
            # keep the PE ticking until the K/Q matmuls start
            warm = psS.tile([1, P], F32, tag="s", name="warm2")
            nc.tensor.matmul(warm, lhsT=zero_sb, rhs=x_sb[:, t, 0:P],
                             start=True, stop=True)

        # ---------------- xn slices + K/Q projections (fp8 DoubleRow) -------
        # Per 512-col slice: normalize both channel halves straight to fp8,
        # then one DR matmul per output half -> bias-add -> fp8 store.
        q_sb = big.tile([P, CT, N], FP8, name="q_sb")   # (c_half, n)
        k_sb = big.tile([P, CT, N], FP8, name="k_sb")
        vT_sb = big.tile([P, MT, C], FP8, name="vT_sb")  # (n, c), n on partitions

        for s in range(NNCH):
            nsl = ds(s * NCH, NCH)
            for t in range(CT):
                eng = nc.vector if (s * CT + t) % 2 == 0 else nc.gpsimd
                eng.tensor_scalar(out=xn_sb[:, t, nsl], in0=x_sb[:, t, nsl],
                                  scalar1=ab[:, t, 0:1], scalar2=ab[:, t, 1:2],
                                  op0=OP.mult, op1=OP.add)

        qki = 0
        for (w_sb, bcol, o_sb) in ((wk_sb, BK, k_sb), (wq_sb, BQ, q_sb)):
            for s in range(NNCH):
                nsl = ds(s * NCH, NCH)
                pp = psS.tile([P, CT, NCH], F32, tag="s", name="ppqk")
                for tq in range(CT):
                    nc.tensor.matmul(pp[:, tq, :],
                                     lhsT=w_sb[:, :, ts(tq, P)],
                                     rhs=xn_sb[:, :, nsl],
                                     start=True, stop=True, perf_mode=DR)
                for tq in range(CT):
                    osl = o_sb[:, tq, nsl]
                    if qki % 2 == 0:
                        nc.vector.tensor_scalar_add(out=osl, in0=pp[:, tq, :],
                                                    scalar1=ca[:, bcol + tq:bcol + tq + 1])
                    else:
                        nc.scalar.activation(out=osl, in_=pp[:, tq, :],
                                             func=AF.Identity,
                                             bias=ca[:, bcol + tq:bcol + tq + 1])
                    qki += 1

        # ---------------- attention + output projection (fp8 DoubleRow) -----
        # v-tile pairs are produced inside nch 0's stream (PSUM bank shared
        # with proj; b_v is folded into b_out host-side).
        LAG = 2                      # AV/den trail the scores/exp stream
        for nch in range(NNCH):
            nsl = ds(nch * NCH, NCH)
            attn = psA.tile([P, CT, NCH], F32, tag="attn", name="attn")
            den = psD.tile([1, NCH], F32, tag="den", name="den")
            es = {}

            def _av(mp):
                e = es.pop(mp)
                for ch in range(CT):
                    nc.tensor.matmul(attn[:, ch, :],
                                     lhsT=vT_sb[:, ds(2 * mp, 2), ts(ch, P)],
                                     rhs=e,
                                     start=(mp == 0), stop=(mp == MP - 1),
                                     perf_mode=DR)
                nc.tensor.matmul(den, lhsT=ones_sb[:, :, 0:1], rhs=e,
                                 start=(mp == 0), stop=(mp == MP - 1),
                                 perf_mode=DR)

            for mp in range(MP):
                if nch == 0:
                    vt = psP.tile([P, CT, C], F32, tag="proj", name="vt")
                    for half in range(2):
                        nc.tensor.matmul(vt[:, half, :],
                                         lhsT=xn_sb[:, :, ts(2 * mp + half, P)],
                                         rhs=wv_sb,
                                         start=True, stop=True, perf_mode=DR)
                    nc.vector.tensor_copy(out=vT_sb[:, ds(2 * mp, 2), :], in_=vt)
                sp = psS.tile([P, CT, NCH], F32, tag="s", name="sp")
                for half in range(2):
                    nc.tensor.matmul(sp[:, half, :],
                                     lhsT=k_sb[:, :, ts(2 * mp + half, P)],
                                     rhs=q_sb[:, :, nsl],
                                     start=True, stop=True, perf_mode=DR)
                e = work.tile([P, 2, NCH], FP8, tag="e", name="e", bufs=4)
                nc.scalar.activation(out=e, in_=sp, func=AF.Exp,
                                     bias=esh_sb, scale=SCALE)
                es[mp] = e
                if mp >= LAG:
                    _av(mp - LAG)
            for mp in range(MP - LAG, MP):
                _av(mp)

            den_sb = small.tile([1, NCH], F32, tag="den_sb", name="den_sb", bufs=2)
            nc.vector.tensor_copy(out=den_sb, in_=den)
            rden = small.tile([1, NCH], F32, tag="rden", name="rden", bufs=2)
            rscr = small.tile([1, NCH], F32, tag="rscr", name="rscr", bufs=2)
            nc.vector.reciprocal_approx_accurate(out=rden, in_=den_sb, scratch=rscr)
            rdenb = outp.tile([P, NCH], F32, tag="rdenb", name="rdenb", bufs=2)
            nc.gpsimd.partition_broadcast(rdenb, rden)

            atts = outp.tile([P, CT, NCH], F32R, tag="att", name="atts", bufs=2)
            nc.vector.tensor_copy(out=atts, in_=attn)

            # proj per output-channel half on a single PSUM bank; copy to SBUF
            # immediately (PSUM release must not be gated on the rden chain)
            for co in range(CT):
                pj = psP.tile([P, NCH], F32, tag="proj", name="pj")
                for ch in range(CT):
                    nc.tensor.matmul(pj,
                                     lhsT=wo_sb[:, ch, ts(co, P)],
                                     rhs=atts[:, ch, :],
                                     start=(ch == 0), stop=(ch == CT - 1))
                pjs = outp.tile([P, NCH], F32, tag="pjs", name="pjs", bufs=2)
                nc.vector.tensor_copy(out=pjs, in_=pj)
                f = outp.tile([P, NCH], F32, tag="fout", name="f", bufs=2)
                nc.vector.tensor_tensor(out=f, in0=pjs, in1=rdenb, op=OP.mult)
                nc.vector.scalar_tensor_tensor(out=f, in0=f, scalar=ca[:, BO + co:BO + co + 1],
                                               in1=x_sb[:, co, nsl],
                                               op0=OP.add, op1=OP.add)
                nc.sync.dma_start(out=out_d[ts(co, P), nsl], in_=f)


def build_program():
    nc = bacc.Bacc("TRN2", target_bir_lowering=False, debug=False, num_devices=B)
    d = {}

    def din(name, shape, dt_=F32):
        d[name] = nc.dram_tensor(name, list(shape), dt_, kind="ExternalInput").ap()

    din("x", (C, N))
    din("wq8", (P, CT, C), FP8)
    din("wk8", (P, CT, C), FP8)
    din("wv8", (P, CT, C), FP8)
    din("wo_t", (C, C), F32R)
    din("consts_a", (P, 26))
    din("bmask", (G, CT * P))
    out_d = nc.dram_tensor("out", [C, N], F32, kind="ExternalOutput").ap()

    with tile.TileContext(nc) as tc:
        _emit(tc, d, out_d)
    nc.compile()
    return nc


_PROG = None


def _get_program():
    global _PROG
    if _PROG is None:
        _PROG = build_program()
    return _PROG


def make_in_maps(inputs):
    x = np.ascontiguousarray(np.asarray(inputs["x"], dtype=np.float32))
    w_qkv = np.asarray(inputs["w_qkv"], dtype=np.float32)
    b_qkv = np.asarray(inputs["b_qkv"], dtype=np.float32)
    w_out = np.asarray(inputs["w_out"], dtype=np.float32)
    b_out = np.asarray(inputs["b_out"], dtype=np.float32)
    gn_scale = np.asarray(inputs["gn_scale"], dtype=np.float32)
    gn_bias = np.asarray(inputs["gn_bias"], dtype=np.float32)

    fmask = np.zeros((CT, P, G), dtype=np.float32)
    for t in range(CT):
        for p in range(P):
            fmask[t, p, (t * P + p) // GSZ] = 1.0
    # bmask[g, t*P+p] = fmask[t, p, g]
    bmask = np.ascontiguousarray(fmask.transpose(2, 0, 1).reshape(G, CT * P))

    consts_a = np.zeros((P, 26), dtype=np.float32)
    bo_eff = b_out + w_out @ b_qkv[2 * C:3 * C]   # b_v folded (softmax sums to 1)
    for t in range(CT):
        rows = slice(t * P, (t + 1) * P)
        consts_a[:, 0 + t] = b_qkv[0:C][rows]
        consts_a[:, 2 + t] = b_qkv[C:2 * C][rows]
        consts_a[:, 4 + t] = bo_eff[rows]
        consts_a[:, 6 + t] = gn_scale[rows]
        consts_a[:, 8 + t] = gn_bias[rows]
        consts_a[:, 10 + G * t:10 + G * (t + 1)] = fmask[t]

    import ml_dtypes
    E4 = ml_dtypes.float8_e4m3

    def pack8(w):
        # [cout, cin] -> lhsT/rhs pair layout [cin_half, 2, cout] in fp8
        return np.ascontiguousarray(
            w.T.reshape(CT, P, C).transpose(1, 0, 2)).astype(E4)

    common = {
        "wq8": pack8(w_qkv[0:C]),
        "wk8": pack8(w_qkv[C:2 * C]),
        "wv8": pack8(w_qkv[2 * C:3 * C]),
        "wo_t": np.ascontiguousarray(w_out.T),
        "consts_a": consts_a,
        "bmask": bmask,
    }
    return [dict(common, x=np.ascontiguousarray(x[b].reshape(C, N)))
            for b in range(B)]


def run(inputs, trace=False):
    nc = _get_program()
    in_maps = make_in_maps(inputs)
    res = bass_utils.run_bass_kernel_spmd(nc, in_maps, core_ids=list(range(B)),
                                          trace=trace)
    out = np.stack([res.results[b]["out"] for b in range(B)])
    return out.reshape(B, C, HH, WW), res


def kernel(**inputs):
    out, _ = run(inputs, trace=False)
    return out


# revision 25
# speedup vs baseline: 1.3699x; 1.0729x over previous
"""AttentionBlock (GroupNorm + single-head NxN attention + residual) on 8 TRN2 cores.

Data-parallel: batch dim (B=8) sharded 1 batch-image per NeuronCore. Each core
runs the full block for its image:

  x (C=256, N=4096) -> GroupNorm(8 groups) -> qkv = W_qkv @ xn (fp32r) ->
  q,k,v cast to fp8e4m3; attention runs entirely in fp8 DoubleRow matmuls
  (contraction 256 = full C or an m-pair of 128x2 keys in ONE matmul at
  2 MACs/cell/cycle):
    sT = k^T q  (one DR matmul per key-tile), e = exp(sT/16 - 4)  -> fp8
    attnout_u = v @ e (DR, m-pairs), den = ones @ e (DR)
    proj_u = W_out @ attnout_u (fp32r); out = proj_u * (1/den) + b_out + x

The softmax is unnormalized with a constant exponent shift of -4 (keeps
exp() under the fp8e4m3 max of 240; the shift and the 1/den normalization
both commute through the output projection and cancel).

Schedule: the head is pipelined (x DMA chunks -> stats -> xn slices -> k/q
projection per 512-col slice); exp is done once per key-tile PAIR over a
2-bank PSUM tile (scalar engine is the body co-bottleneck); v-tile pairs are
produced inside nch 0's attention stream, sharing a PSUM bank with the
output projection.
"""

import sys

if "/opt/trn_rl_repo" not in sys.path:
    sys.path.insert(0, "/opt/trn_rl_repo")

import numpy as np

import concourse.bass as bass
import concourse.bacc as bacc
import concourse.tile as tile
import concourse.mybir as mybir
from concourse import bass_utils

# Problem dims (hardcoded per spec)
B, C, HH, WW = 8, 256, 64, 64
N = HH * WW            # 4096
G = 8                  # groupnorm groups
GSZ = C // G           # 32 channels/group
EPS = 1e-5
P = 128                # SBUF partitions
CT = C // P            # 2 channel tiles
NCH = 512              # query-chunk width (free dim per matmul)
NNCH = N // NCH        # 8
MT = N // P            # 32 key tiles
MP = MT // 2           # 16 key-tile pairs
SCALE = 1.0 / np.sqrt(C)
ESHIFT = -4.0          # exponent shift; cancels in normalization
INV_CNT = 1.0 / (GSZ * N)

F32 = mybir.dt.float32
F32R = mybir.dt.float32r
FP8 = mybir.dt.float8e4
DR = mybir.MatmulPerfMode.DoubleRow


def _emit(tc, d, out_d):
    from contextlib import ExitStack

    nc = tc.nc
    AF = mybir.ActivationFunctionType
    OP = mybir.AluOpType
    AX = mybir.AxisListType.X
    ts, ds = bass.ts, bass.ds

    with ExitStack() as ctx:
        const = ctx.enter_context(tc.tile_pool(name="const", bufs=1))
        big = ctx.enter_context(tc.tile_pool(name="big", bufs=1))
        work = ctx.enter_context(tc.tile_pool(name="work", bufs=3))
        small = ctx.enter_context(tc.tile_pool(name="small", bufs=4))
        outp = ctx.enter_context(tc.tile_pool(name="outp", bufs=3))
        # PSUM: 8 banks total. s-pair 2 bufs x 2 banks, attn 2 banks,
        # den 1 bank, v/proj shared 1 bank.
        psS = ctx.enter_context(tc.tile_pool(name="psS", bufs=2, space="PSUM"))
        psA = ctx.enter_context(tc.tile_pool(name="psA", bufs=1, space="PSUM"))
        psD = ctx.enter_context(tc.tile_pool(name="psD", bufs=1, space="PSUM"))
        psP = ctx.enter_context(tc.tile_pool(name="psP", bufs=1, space="PSUM"))

        # ---------------- DMAs: packed consts first, then x on 4 queues -----
        # consts_a columns: b_q(2) b_k(2) b_o(2) gn_w(2) gn_b(2) fmask(2x8)
        ca = const.tile([P, 26], F32, name="ca")
        nc.scalar.dma_start(out=ca, in_=d["consts_a"])
        bm_sb = const.tile([G, CT, P], F32, name="bm_sb")
        nc.scalar.dma_start(out=bm_sb[:, :, :], in_=d["bmask"])
        BQ, BK, BO = 0, 2, 4         # ca column offsets

        # x on the sync/gpsimd queues only (the scalar queue carries the small
        # consts + fp8 weights and must stay clear for the GN squares)
        NC4 = 4                      # head chunks per channel-tile
        CW = N // NC4                # 1024 columns per chunk
        x_d = d["x"]
        x_sb = big.tile([P, CT, N], F32, name="x_sb")
        for c in range(NC4):
            for t in range(CT):
                csl = ds(c * CW, CW)
                eng = nc.sync if t == 0 else nc.gpsimd
                eng.dma_start(out=x_sb[:, t, csl], in_=x_d[ts(t, P), csl])

        # fp8 pair-packed weights ([cin_half, 2, cout])
        wq_sb = const.tile([P, CT, C], FP8, name="wq_sb")
        wk_sb = const.tile([P, CT, C], FP8, name="wk_sb")
        wv_sb = const.tile([P, CT, C], FP8, name="wv_sb")
        wo_sb = const.tile([P, CT, C], FP8, name="wo_sb")
        for sb, dr in ((wk_sb, d["wk8"]), (wq_sb, d["wq8"]),
                       (wv_sb, d["wv8"]), (wo_sb, d["wo8"])):
            nc.scalar.dma_start(out=sb, in_=dr)

        # fp8 ones pair for the den matmul (pair-dim stride kept at 16B)
        ones_sb = const.tile([P, CT, 16], FP8, name="ones_sb")
        nc.gpsimd.memset(ones_sb, 1.0)
        zero_sb = const.tile([P, 1], F32, name="zero_sb")
        nc.gpsimd.memset(zero_sb, 0.0)
        esh_sb = const.tile([P, 1], F32, name="esh_sb")
        nc.gpsimd.memset(esh_sb, ESHIFT)
        eps_sb = const.tile([G, 1], F32, name="eps_sb")
        nc.gpsimd.memset(eps_sb, EPS)

        # ---------------- GroupNorm stats (pipelined behind x DMA) ----------
        # Per chunk: row-sums on vector, x^2 row-sums on scalar (accum_out;
        # the squared values land in a throwaway f32 scratch); group-combine
        # matmul accumulates as chunks complete.
        xn_sb = big.tile([P, CT, N], FP8, name="xn_sb")
        sq_scr = small.tile([P, 2, CW], F32, name="sq_scr")
        stat = small.tile([P, CT, NC4, 2], F32, name="stat")
        gps = psS.tile([G, 2], F32, tag="s", name="gps")
        for c in range(NC4):
            for t in range(CT):
                csl = ds(c * CW, CW)
                nc.vector.reduce_sum(out=stat[:, t, c, 0:1], in_=x_sb[:, t, csl],
                                     axis=AX)
                nc.scalar.activation(out=sq_scr[:, (c * CT + t) % 2, :],
                                     in_=x_sb[:, t, csl],
                                     func=AF.Square, bias=zero_sb,
                                     accum_out=stat[:, t, c, 1:2])
                # PE warm-up during the head: short fp32 matmul on the chunk
                if t == 0:
                    warm = psS.tile([1, P], F32, tag="s", name="warm")
                    nc.tensor.matmul(warm, lhsT=zero_sb,
                                     rhs=x_sb[:, t, ds(c * CW, P)],
                                     start=True, stop=True)
            for t in range(CT):
                nc.tensor.matmul(gps, lhsT=ca[:, ds(10 + G * t, G)],
                                 rhs=stat[:, t, c, :],
                                 start=(c == 0 and t == 0),
                                 stop=(c == NC4 - 1 and t == CT - 1))
        grp = small.tile([G, 2], F32, name="grp")    # [mean, rstd]
        gtmp = small.tile([G, 3], F32, name="gtmp")
        nc.vector.tensor_scalar_mul(out=grp[:, 0:1], in0=gps[:, 0:1], scalar1=INV_CNT)
        nc.vector.tensor_scalar_mul(out=gtmp[:, 0:1], in0=gps[:, 1:2], scalar1=INV_CNT)
        nc.vector.tensor_mul(out=gtmp[:, 1:2], in0=grp[:, 0:1], in1=grp[:, 0:1])
        nc.vector.tensor_sub(out=gtmp[:, 2:3], in0=gtmp[:, 0:1], in1=gtmp[:, 1:2])
        nc.scalar.activation(out=gtmp[:, 2:3], in_=gtmp[:, 2:3], func=AF.Sqrt,
                             bias=eps_sb)
        nc.vector.reciprocal(out=grp[:, 1:2], in_=gtmp[:, 2:3])

        ab = small.tile([P, CT, 2], F32, name="ab")  # per-channel scale a, bias b
        for t in range(CT):
            cps = psS.tile([P, 2], F32, tag="s", name="cps")
            nc.tensor.matmul(cps, lhsT=bm_sb[:, t, :], rhs=grp, start=True, stop=True)
            nc.vector.tensor_mul(out=ab[:, t, 0:1], in0=cps[:, 1:2], in1=ca[:, 6 + t:7 + t])
            nc.vector.tensor_mul(out=ab[:, t, 1:2], in0=cps[:, 0:1], in1=ab[:, t, 0:1])
            nc.vector.tensor_sub(out=ab[:, t, 1:2], in0=ca[:, 8 + t:9 + t], in1=ab[:, t, 1:2])
            # keep the PE ticking until the K/Q matmuls start
            warm = psS.tile([1, P], F32, tag="s", name="warm2")
            nc.tensor.matmul(warm, lhsT=zero_sb, rhs=x_sb[:, t, 0:P],
                             start=True, stop=True)

        # ---------------- xn slices + K/Q projections (fp8 DoubleRow) -------
        # Per 512-col slice: normalize both channel halves straight to fp8,
        # then one DR matmul per output half -> bias-add -> fp8 store.
        q_sb = big.tile([P, CT, N], FP8, name="q_sb")   # (c_half, n)
        k_sb = big.tile([P, CT, N], FP8, name="k_sb")
        vT_sb = big.tile([P, MT, C], FP8, name="vT_sb")  # (n, c), n on partitions

        for s in range(NNCH):
            nsl = ds(s * NCH, NCH)
            for t in range(CT):
                eng = nc.vector if (s * CT + t) % 2 == 0 else nc.gpsimd
                eng.tensor_scalar(out=xn_sb[:, t, nsl], in0=x_sb[:, t, nsl],
                                  scalar1=ab[:, t, 0:1], scalar2=ab[:, t, 1:2],
                                  op0=OP.mult, op1=OP.add)

        qki = 0
        for (w_sb, bcol, o_sb) in ((wk_sb, BK, k_sb), (wq_sb, BQ, q_sb)):
            for s in range(NNCH):
                nsl = ds(s * NCH, NCH)
                pp = psS.tile([P, CT, NCH], F32, tag="s", name="ppqk")
                for tq in range(CT):
                    nc.tensor.matmul(pp[:, tq, :],
                                     lhsT=w_sb[:, :, ts(tq, P)],
                                     rhs=xn_sb[:, :, nsl],
                                     start=True, stop=True, perf_mode=DR)
                for tq in range(CT):
                    osl = o_sb[:, tq, nsl]
                    if qki % 2 == 0:
                        nc.vector.tensor_scalar_add(out=osl, in0=pp[:, tq, :],
                                                    scalar1=ca[:, bcol + tq:bcol + tq + 1])
                    else:
                        nc.scalar.activation(out=osl, in_=pp[:, tq, :],
                                             func=AF.Identity,
                                             bias=ca[:, bcol + tq:bcol + tq + 1])
                    qki += 1
        # preload the Exp activation table before the body needs it
        nc.scalar.activation(out=gtmp[:, 0:1], in_=eps_sb, func=AF.Exp,
                             bias=eps_sb)

        # ---------------- attention + output projection (fp8 DoubleRow) -----
        # v-tile pairs are produced inside nch 0's stream (PSUM bank shared
        # with proj; b_v is folded into b_out host-side).
        # Flat (nch, mp) stream; AV/den trail the scores/exp stream by LAG
        # units ACROSS nch boundaries so the per-nch PSUM handoff (attn CAST,
        # den copy) hides under the next nch's score matmuls.
        LAG = 2
        attns, dens = {}, {}

        def _tail(nch):
            nsl = ds(nch * NCH, NCH)
            den_sb = small.tile([1, NCH], F32, tag="den_sb", name="den_sb", bufs=2)
            nc.vector.tensor_copy(out=den_sb, in_=dens.pop(nch))
            rden = small.tile([1, NCH], F32, tag="rden", name="rden", bufs=2)
            nc.vector.reciprocal_approx_fast(out=rden, in_=den_sb)
            rdenb = outp.tile([P, NCH], F32, tag="rdenb", name="rdenb", bufs=2)
            nc.gpsimd.partition_broadcast(rdenb, rden)

            atts = outp.tile([P, CT, NCH], FP8, tag="att", name="atts", bufs=2)
            nc.vector.tensor_copy(out=atts, in_=attns.pop(nch))

            # proj per output-channel half on a single PSUM bank; copy to SBUF
            # immediately (PSUM release must not be gated on the rden chain)
            for co in range(CT):
                pj = psP.tile([P, NCH], F32, tag="proj", name="pj")
                nc.tensor.matmul(pj, lhsT=wo_sb[:, :, ts(co, P)], rhs=atts,
                                 start=True, stop=True, perf_mode=DR)
                pjs = outp.tile([P, NCH], F32, tag="pjs", name="pjs", bufs=2)
                nc.vector.tensor_copy(out=pjs, in_=pj)
                f = outp.tile([P, NCH], F32, tag="fout", name="f", bufs=2)
                nc.vector.tensor_tensor(out=f, in0=pjs, in1=rdenb, op=OP.mult)
                nc.vector.scalar_tensor_tensor(out=f, in0=f, scalar=ca[:, BO + co:BO + co + 1],
                                               in1=x_sb[:, co, nsl],
                                               op0=OP.add, op1=OP.add)
                nc.sync.dma_start(out=out_d[ts(co, P), nsl], in_=f)

        def _av(nch, mp, e):
            if mp == 0:
                attns[nch] = psA.tile([P, CT, NCH], F32, tag="attn", name="attn")
                dens[nch] = psD.tile([1, NCH], F32, tag="den", name="den")
            for ch in range(CT):
                nc.tensor.matmul(attns[nch][:, ch, :],
                                 lhsT=vT_sb[:, ds(2 * mp, 2), ts(ch, P)],
                                 rhs=e,
                                 start=(mp == 0), stop=(mp == MP - 1),
                                 perf_mode=DR)
            nc.tensor.matmul(dens[nch], lhsT=ones_sb[:, :, 0:1], rhs=e,
                             start=(mp == 0), stop=(mp == MP - 1),
                             perf_mode=DR)
            if mp == MP - 1:
                _tail(nch)

        pend = []
        for nch in range(NNCH):
            nsl = ds(nch * NCH, NCH)
            for mp in range(MP):
                if nch == 0:
                    vt = psP.tile([P, CT, C], F32, tag="proj", name="vt")
                    for half in range(2):
                        nc.tensor.matmul(vt[:, half, :],
                                         lhsT=xn_sb[:, :, ts(2 * mp + half, P)],
                                         rhs=wv_sb,
                                         start=True, stop=True, perf_mode=DR)
                    nc.vector.tensor_copy(out=vT_sb[:, ds(2 * mp, 2), :], in_=vt)
                sp = psS.tile([P, CT, NCH], F32, tag="s", name="sp")
                for half in range(2):
                    nc.tensor.matmul(sp[:, half, :],
                                     lhsT=k_sb[:, :, ts(2 * mp + half, P)],
                                     rhs=q_sb[:, :, nsl],
                                     start=True, stop=True, perf_mode=DR)
                e = work.tile([P, 2, NCH], FP8, tag="e", name="e", bufs=4)
                nc.scalar.activation(out=e, in_=sp, func=AF.Exp,
                                     bias=esh_sb, scale=SCALE)
                pend.append((nch, mp, e))
                if len(pend) > LAG:
                    _av(*pend.pop(0))
        for unit in pend:
            _av(*unit)


def build_program():
    nc = bacc.Bacc("TRN2", target_bir_lowering=False, debug=False, num_devices=B)
    d = {}

    def din(name, shape, dt_=F32):
        d[name] = nc.dram_tensor(name, list(shape), dt_, kind="ExternalInput").ap()

    din("x", (C, N))
    din("wq8", (P, CT, C), FP8)
    din("wk8", (P, CT, C), FP8)
    din("wv8", (P, CT, C), FP8)
    din("wo8", (P, CT, C), FP8)
    din("consts_a", (P, 26))
    din("bmask", (G, CT * P))
    out_d = nc.dram_tensor("out", [C, N], F32, kind="ExternalOutput").ap()

    with tile.TileContext(nc) as tc:
        _emit(tc, d, out_d)
    nc.compile()
    return nc


_PROG = None


def _get_program():
    global _PROG
    if _PROG is None:
        _PROG = build_program()
    return _PROG


def make_in_maps(inputs):
    x = np.ascontiguousarray(np.asarray(inputs["x"], dtype=np.float32))
    w_qkv = np.asarray(inputs["w_qkv"], dtype=np.float32)
    b_qkv = np.asarray(inputs["b_qkv"], dtype=np.float32)
    w_out = np.asarray(inputs["w_out"], dtype=np.float32)
    b_out = np.asarray(inputs["b_out"], dtype=np.float32)
    gn_scale = np.asarray(inputs["gn_scale"], dtype=np.float32)
    gn_bias = np.asarray(inputs["gn_bias"], dtype=np.float32)

    fmask = np.zeros((CT, P, G), dtype=np.float32)
    for t in range(CT):
        for p in range(P):
            fmask[t, p, (t * P + p) // GSZ] = 1.0
    # bmask[g, t*P+p] = fmask[t, p, g]
    bmask = np.ascontiguousarray(fmask.transpose(2, 0, 1).reshape(G, CT * P))

    consts_a = np.zeros((P, 26), dtype=np.float32)
    bo_eff = b_out + w_out @ b_qkv[2 * C:3 * C]   # b_v folded (softmax sums to 1)
    for t in range(CT):
        rows = slice(t * P, (t + 1) * P)
        consts_a[:, 0 + t] = b_qkv[0:C][rows]
        consts_a[:, 2 + t] = b_qkv[C:2 * C][rows]
        consts_a[:, 4 + t] = bo_eff[rows]
        consts_a[:, 6 + t] = gn_scale[rows]
        consts_a[:, 8 + t] = gn_bias[rows]
        consts_a[:, 10 + G * t:10 + G * (t + 1)] = fmask[t]

    import ml_dtypes
    E4 = ml_dtypes.float8_e4m3

    def pack8(w):
        # [cout, cin] -> lhsT/rhs pair layout [cin_half, 2, cout] in fp8
        return np.ascontiguousarray(
            w.T.reshape(CT, P, C).transpose(1, 0, 2)).astype(E4)

    common = {
        "wq8": pack8(w_qkv[0:C]),
        "wk8": pack8(w_qkv[C:2 * C]),
        "wv8": pack8(w_qkv[2 * C:3 * C]),
        "wo8": pack8(w_out),
        "consts_a": consts_a,
        "bmask": bmask,
    }
    return [dict(common, x=np.ascontiguousarray(x[b].reshape(C, N)))
            for b in range(B)]


def run(inputs, trace=False):
    nc = _get_program()
    in_maps = make_in_maps(inputs)
    res = bass_utils.run_bass_kernel_spmd(nc, in_maps, core_ids=list(range(B)),
                                          trace=trace)
    out = np.stack([res.results[b]["out"] for b in range(B)])
    return out.reshape(B, C, HH, WW), res


def kernel(**inputs):
    out, _ = run(inputs, trace=False)
    return out
